# revision 40
# baseline (speedup 1.0000x reference)
"""Multi-head attention (B=4, N=2048, C=768, H=12) on 8 Trainium2 NeuronCores.

Sharding: core c = (batch b = c//2, head-group g = c%2 of 6 heads).
Each core: qkv projection for its (b, g), attention for 6 heads, partial
output projection against w_proj[:, g-cols]. Host sums the two partial
projections per batch, adds bias, transposes. No collectives.

v3 design: ACT exp stream (192 x [128,1024] activations ~= 199us) is the
bottleneck; PE work is ~185us thanks to transposed-PV (out [q=128, 65]
costs 65 rows/matmul instead of 512+). Everything is scheduled to keep
ACT streaming:
  - diagonal segment order (pair, qgroup) so early segments only need
    pair0's q/k/v and projections spread evenly
  - all deferred PE work (qkv pieces, v-chunks, proj) split into
    ~160-320ns units, deadline-paced 1-2 per chunk slot
  - all inputs bf16, loads n-ordered and fine-grained at the start
  - softmax denominators per-partition after transposed PV: normalize is
    reciprocal + tensor_scalar on DVE; [q,d]->[d,q] via dma transpose
  - last q-group's projection pre-accumulates fc0/fc1 into SBUF so the
    tail after the final normalize is one fc2 matmul + add per piece
PSUM (8 banks): scores [128,1024]x2 (4) + PV [128,512]x2 (2) + qkv/proj
[128,<=512]x2 (2).

Attention per segment (pair p, qgroup qg of 512 q), chunk ch (128 k):
  sT[k, q] e0|e1 -> one [128,1024] psum tile; one exp -> eT bf16 sbuf
  PV per (e, qb): lhsT = eT[:, e*512+qb*128 :+128], rhs = vT[:, ch, h*65 :+65]
  (col 64 of vT == 1 -> denominator lands at out[q, 64])
"""

import sys

for _p in ("/opt/trn_rl_repo", "/root/.axon_site/_ro/trn_rl_repo"):
    if _p not in sys.path:
        sys.path.insert(0, _p)

import numpy as np
import ml_dtypes

import concourse.bass as bass
import concourse.bacc as bacc
import concourse.mybir as mybir
import concourse.tile as tile
from concourse.bass_utils import run_bass_kernel_spmd

B, N, C = 4, 2048, 768
H, D = 12, 64
HG = 6          # heads per core
P = 128
NCORES = 8
KC = C // P     # 6 contraction chunks for qkv
NT = N // P     # 16 k-chunks of 128
NP_ = 3         # head pairs per core
QGW = 512       # q-group width
NQG = N // QGW  # 4
QB = QGW // P   # 4 q-blocks of 128 per q-group
FC = HG * D // P  # 3 proj contraction blocks
SCALE = D ** -0.5

B_DT = mybir.dt.bfloat16
NP_BF = ml_dtypes.bfloat16

# diagonal (pair, qgroup) order: early segments only need pair0 inputs,
# q-groups complete progressively so proj work spreads out
SEG_ORDER = [
    (0, 0), (0, 1), (1, 0), (0, 2), (1, 1), (2, 0),
    (0, 3), (1, 2), (2, 1), (1, 3), (2, 2), (2, 3),
]

_CACHED_NC = None
DEBUG_DUMPS = False


def build_nc():
    nc = bacc.Bacc("TRN2", target_bir_lowering=False, debug=False, num_devices=NCORES)
    f32 = mybir.dt.float32

    xT = nc.declare_dram_parameter("xT", [P, KC, N], B_DT, isOutput=False)
    wqk = nc.declare_dram_parameter("wqk", [P, KC, 2 * HG * D], B_DT, isOutput=False)
    wv = nc.declare_dram_parameter("wv", [P, KC, HG * D], B_DT, isOutput=False)
    wp = nc.declare_dram_parameter("wp", [P, FC, C], B_DT, isOutput=False)
    out = nc.declare_dram_parameter("out", [C, N], B_DT, isOutput=True)
    if DEBUG_DUMPS:
        dbg_qk = nc.declare_dram_parameter("dbg_qk", [P, 2 * HG * D // P, N], B_DT, isOutput=True)
        dbg_vt = nc.declare_dram_parameter("dbg_vt", [P, NT, HG * (D + 1)], B_DT, isOutput=True)
        dbg_outh = nc.declare_dram_parameter("dbg_outh", [P, FC, N], B_DT, isOutput=True)
        dbg_et = nc.declare_dram_parameter("dbg_et", [P, 2 * QGW], B_DT, isOutput=True)
        dbg_pv = nc.declare_dram_parameter("dbg_pv", [P, 512], mybir.dt.float32, isOutput=True)

    with tile.TileContext(nc) as tc:
        with (
            tc.tile_pool(name="big", bufs=1) as big,
            tc.tile_pool(name="et", bufs=10) as etp,
            tc.tile_pool(name="nrm", bufs=8) as nrm,
            tc.tile_pool(name="stg", bufs=4) as stgp,
            tc.tile_pool(name="so", bufs=8) as sop,
            tc.tile_pool(name="psS", bufs=2, space="PSUM") as psS,
            tc.tile_pool(name="psPV", bufs=2, space="PSUM") as psPV,
            tc.tile_pool(name="psC", bufs=2, space="PSUM") as psC,
        ):
            xT_sb = big.tile([P, KC, N], B_DT)
            wqk_sb = big.tile([P, KC, 2 * HG * D], B_DT)
            wv_sb = big.tile([P, KC, HG * D], B_DT)
            wp_sb = big.tile([P, FC, C], B_DT)

            # ---- loads: first-exp critical path first, fine-grained ----
            nc.sync.dma_start(wqk_sb[:, :, 0:P], wqk[:, :, 0:P])              # q pair0
            nc.scalar.dma_start(wqk_sb[:, :, 3 * P : 4 * P], wqk[:, :, 3 * P : 4 * P])  # k pair0
            for kc in range(KC):  # tokens 0:512 per contraction chunk
                eng = nc.sync if kc % 2 == 0 else nc.scalar
                eng.dma_start(xT_sb[:, kc, 0:QGW], xT[:, kc, 0:QGW])
            nc.scalar.dma_start(wv_sb, wv[:, :, :])
            for npc in range(1, NQG):
                nc.sync.dma_start(
                    xT_sb[:, :, npc * QGW : (npc + 1) * QGW],
                    xT[:, :, npc * QGW : (npc + 1) * QGW],
                )
            nc.sync.dma_start(wqk_sb[:, :, P : 3 * P], wqk[:, :, P : 3 * P])
            nc.sync.dma_start(wqk_sb[:, :, 4 * P :], wqk[:, :, 4 * P :])
            nc.sync.dma_start(wp_sb, wp[:, :, :])

            # warm the ACT exp table during the load phase
            warm = nrm.tile([1, 32], f32, tag="warm")
            nc.vector.memset(warm, 0.0)
            nc.scalar.activation(warm, warm, mybir.ActivationFunctionType.Exp,
                                 bias=0.0, scale=1.0)
            # warm the PE p-state: junk matmuls bridge the input-load latency
            # so real work starts at full clock with no ramp resets
            junk = big.tile([P, D], B_DT, name="junk")
            nc.vector.memset(junk, 0.0)
            junk_ps = psC.tile([D, D], f32, tag="c", name="junk_ps")
            for _ in range(100):
                nc.tensor.matmul(junk_ps, lhsT=junk, rhs=junk, start=True, stop=True)
            # 128x128 identity (bf16) for PE-side transposes in the endgame
            io_a = big.tile([P, P], mybir.dt.float32, name="io_a")
            io_b = big.tile([P, 1], mybir.dt.float32, name="io_b")
            nc.gpsimd.iota(io_a, pattern=[[1, P]], base=0, channel_multiplier=0,
                           allow_small_or_imprecise_dtypes=True)
            nc.gpsimd.iota(io_b, pattern=[[0, 1]], base=0, channel_multiplier=1,
                           allow_small_or_imprecise_dtypes=True)
            id_sb = big.tile([P, P], B_DT, name="id_sb")
            nc.vector.tensor_scalar(id_sb, io_a, io_b[:, 0:1], None,
                                    op0=mybir.AluOpType.is_equal)

            qk_sb = big.tile([P, 2 * HG * D // P, N], B_DT)
            vT_sb = big.tile([P, NT, HG * (D + 1)], B_DT)
            ones_view = vT_sb.rearrange("p n (h s) -> p n h s", s=D + 1)[:, :, :, D : D + 1]
            nc.vector.memset(ones_view, 1.0)
            outh_sb = big.tile([P, FC, N], B_DT)

            # ---------------- deferred work units (~160-320 ns each) -----
            def u_qkv(blk, nf):
                """qk_sb[:, blk, nf*128:+128] (128-token piece, 6 matmuls)"""
                def go():
                    ps = psC.tile([P, P], f32, tag="c", name=f"qkv_ps{blk}_{nf}")
                    for kc in range(KC):
                        nc.tensor.matmul(
                            ps,
                            lhsT=wqk_sb[:, kc, blk * P : (blk + 1) * P],
                            rhs=xT_sb[:, kc, nf * P : (nf + 1) * P],
                            start=(kc == 0),
                            stop=(kc == KC - 1),
                        )
                    nc.vector.tensor_copy(qk_sb[:, blk, nf * P : (nf + 1) * P], ps)
                return (0.32, go, 0)

            def u_vt(p_, nt):
                """v rows for pair p_, token chunk nt -> vT_sb (6 matmuls)"""
                def go():
                    ps = psC.tile([P, 2 * D], f32, tag="c", name=f"vt_ps{p_}_{nt}")
                    for kc in range(KC):
                        nc.tensor.matmul(
                            ps,
                            lhsT=xT_sb[:, kc, nt * P : (nt + 1) * P],
                            rhs=wv_sb[:, kc, 2 * p_ * D : 2 * (p_ + 1) * D],
                            start=(kc == 0),
                            stop=(kc == KC - 1),
                        )
                    nc.vector.tensor_copy(
                        vT_sb.rearrange("p n (h s) -> p n h s", s=D + 1)[
                            :, nt, 2 * p_ : 2 * (p_ + 1), 0:D
                        ],
                        ps.rearrange("p (h s) -> p h s", s=D),
                    )
                return (0.32, go, 0)

            so_tiles = {}


            def u_proj(ot, qg, qb, min_ch=0):
                """projection piece [128, 128]: all 3 fc blocks + stage"""
                def go():
                    key = (ot, qg)
                    if key not in so_tiles:
                        so_tiles[key] = sop.tile(
                            [P, QGW], B_DT, tag="so", name=f"so{ot}_{qg}"
                        )
                    so = so_tiles[key]
                    ps = psC.tile([P, P], f32, tag="c", name=f"pj_ps{ot}_{qg}_{qb}")
                    gcols = slice(qg * QGW + qb * P, qg * QGW + (qb + 1) * P)
                    for fc in range(FC):
                        nc.tensor.matmul(
                            ps,
                            lhsT=wp_sb[:, fc, ot * P : (ot + 1) * P],
                            rhs=outh_sb[:, fc, gcols],
                            start=(fc == 0),
                            stop=(fc == FC - 1),
                        )
                    nc.vector.tensor_copy(so[:, qb * P : (qb + 1) * P], ps)
                    if qb == QB - 1:
                        nc.sync.dma_start(
                            out[ot * P : (ot + 1) * P, qg * QGW : (qg + 1) * QGW], so
                        )
                return (0.16, go, min_ch)

            # ---------------- attention pieces ----------------
            et_tiles = {}

            def emit_scores(p_, qg, ch):
                sc = psS.tile([P, 2 * QGW], f32, tag="s", name=f"sc{p_}_{qg}_{ch}")
                for e in range(2):
                    base = e * D
                    nc.tensor.matmul(
                        sc[:, e * QGW : (e + 1) * QGW],
                        lhsT=qk_sb[base : base + D, NP_ + p_, ch * P : (ch + 1) * P],
                        rhs=qk_sb[base : base + D, p_, qg * QGW : (qg + 1) * QGW],
                        start=True,
                        stop=True,
                    )
                eT = etp.tile([P, 2 * QGW], B_DT, tag="et", name=f"et{p_}_{qg}_{ch}")
                nc.scalar.activation(
                    eT, sc, mybir.ActivationFunctionType.Exp,
                    bias=0.0, scale=float(SCALE),
                )
                if DEBUG_DUMPS and (p_, qg, ch) == (0, 0, 0):
                    nc.sync.dma_start(dbg_et[:, :], eT)
                et_tiles[(p_, qg, ch)] = eT

            def emit_pv(p_, qg, ch, pvt):
                # one psum zero-region (bank) per pvt tile: only the first
                # region write starts the group, only the last stops it
                eT = et_tiles.pop((p_, qg, ch))
                for e in range(2):
                    h = 2 * p_ + e
                    for qb in range(QB):
                        t, sub = divmod(qb, 2)
                        nc.tensor.matmul(
                            pvt[t][:, sub * 256 + e * (D + 1) : sub * 256 + (e + 1) * (D + 1)],
                            lhsT=eT[:, e * QGW + qb * P : e * QGW + (qb + 1) * P],
                            rhs=vT_sb[:, ch, h * (D + 1) : (h + 1) * (D + 1)],
                            start=(ch == 0 and e == 0 and sub == 0),
                            stop=(ch == NT - 1 and e == 1 and sub == 1),
                            skip_group_check=True,
                        )

            def emit_normalize(p_, qg, qb, pvt, stg, use_act=False):
                t, sub = divmod(qb, 2)
                seg_tile = pvt[t][:, sub * 256 : sub * 256 + 2 * (D + 1)]
                den = seg_tile.rearrange("p (e s) -> p e s", s=D + 1)[:, :, D]
                rc = nrm.tile([P, 2], f32, tag="rc", name=f"rc{p_}_{qg}_{qb}")
                nc.vector.reciprocal(rc, den)
                for e in range(2):
                    if use_act and e == 0:
                        nc.scalar.activation(
                            stg[:, qb, e * D : (e + 1) * D],
                            seg_tile[:, e * (D + 1) : e * (D + 1) + D],
                            mybir.ActivationFunctionType.Copy,
                            bias=0.0, scale=rc[:, e : e + 1],
                        )
                    else:
                        nc.vector.tensor_scalar_mul(
                            stg[:, qb, e * D : (e + 1) * D],
                            seg_tile[:, e * (D + 1) : e * (D + 1) + D],
                            rc[:, e : e + 1],
                        )

            # ---------------- deadline-paced filler schedule --------------
            due = {si: [] for si in range(len(SEG_ORDER))}
            # seg0: k(p0) token pieces n1..15 + vt(p0) interleaved slot-wise,
            # then q(p0,qg1)
            s0 = [u_qkv(NP_ + 0, i) for i in range(1, 4)]
            s0.append(u_qkv(0, 4))                               # q(p0,qg1) piece
            for i in range(NT):
                s0.append(u_vt(0, i))
                if 4 + i < NT:
                    s0.append(u_qkv(NP_ + 0, 4 + i))
            s0 += [u_qkv(0, 5 + i) for i in range(3)]            # q(p0,qg1) rest
            due[0] = s0
            due[1] = (
                [u_qkv(1, i) for i in range(4)]                  # q(p1,qg0)
                + [u_qkv(NP_ + 1, i) for i in range(NT)]         # k(p1)
                + [u_vt(1, 0)]
            )
            due[2] = (
                [u_vt(1, i) for i in range(1, NT)]
                + [u_qkv(0, 8 + i) for i in range(4)]            # q(p0,qg2)
            )
            due[3] = (
                [u_qkv(1, 4 + i) for i in range(4)]              # q(p1,qg1)
                + [u_qkv(2, i) for i in range(4)]                # q(p2,qg0)
                + [u_qkv(NP_ + 2, i) for i in range(NT)]         # k(p2)
            )
            due[4] = (
                [u_vt(2, i) for i in range(NT)]
                + [u_qkv(0, 12 + i) for i in range(4)]           # q(p0,qg3)
            )
            due[5] = (
                [u_qkv(1, 8 + i) for i in range(4)]              # q(p1,qg2)
                + [u_qkv(2, 4 + i) for i in range(4)]            # q(p2,qg1)
            )
            due[6] = (
                [u_qkv(1, 12 + i) for i in range(4)]             # q(p1,qg3)
                + [u_proj(ot, 0, qb, 4) for ot in range(3) for qb in range(QB)]
            )
            due[7] = (
                [u_qkv(2, 8 + i) for i in range(4)]              # q(p2,qg2)
                + [u_proj(ot, 0, qb) for ot in range(3, 6) for qb in range(QB)]
            )
            due[8] = [u_qkv(2, 12 + i) for i in range(4)]        # q(p2,qg3)
            due[9] = [u_proj(ot, 1, qb, 4) for ot in range(3) for qb in range(QB)]
            due[10] = [u_proj(ot, 1, qb) for ot in range(3, 6) for qb in range(QB)]
            due[11] = [u_proj(ot, 2, qb, 4) for ot in range(6) for qb in range(QB)]

            # ---------------- upfront: q(p0,qg0) + k(p0,ch0) --------------
            ps_q = psC.tile([P, QGW], f32, tag="c", name="up_q")
            ps_k = psC.tile([P, P], f32, tag="c", name="up_k")
            for kc in range(KC):
                nc.tensor.matmul(
                    ps_q,
                    lhsT=wqk_sb[:, kc, 0:P],
                    rhs=xT_sb[:, kc, 0:QGW],
                    start=(kc == 0),
                    stop=(kc == KC - 1),
                )
                nc.tensor.matmul(
                    ps_k,
                    lhsT=wqk_sb[:, kc, NP_ * P : (NP_ + 1) * P],
                    rhs=xT_sb[:, kc, 0:P],
                    start=(kc == 0),
                    stop=(kc == KC - 1),
                )
            nc.vector.tensor_copy(qk_sb[:, 0, 0:QGW], ps_q)
            nc.scalar.copy(qk_sb[:, NP_, 0:P], ps_k)

            # ---------------- main loop ----------------
            pending_end = [None]

            def finish_segment(phase=2):
                """Previous segment's endgame, split across the next segment's
                first two chunk slots so the ACT stream never waits on
                boundary work."""
                if pending_end[0] is None:
                    return
                pp, pq, ppvt, pstg, plag = pending_end[0]
                if phase == 0:
                    for c in range(NT - plag, NT - 1):
                        emit_pv(pp, pq, c, ppvt)
                    return
                pending_end[0] = None
                if phase == 2:
                    for c in range(NT - plag, NT - 1):
                        emit_pv(pp, pq, c, ppvt)
                emit_pv(pp, pq, NT - 1, ppvt)
                if DEBUG_DUMPS and (pp, pq) == (0, 0):
                    pvstg = big.tile([P, 512], f32, name="pvstg")
                    nc.vector.tensor_copy(pvstg, ppvt[0])
                    nc.sync.dma_start(dbg_pv[:, :], pvstg)
                if (pp, pq) != (2, 3):
                    for qb in range(QB):
                        emit_normalize(pp, pq, qb, ppvt, pstg)
                    nc.sync.dma_start_transpose(
                        outh_sb[:, pp, pq * QGW : (pq + 1) * QGW].rearrange(
                            "p (b q) -> p b q", q=P
                        ),
                        pstg.rearrange("p b f -> p (b f)"),
                    )
                else:
                    # endgame: PE-side transposes (no DMA latency), per-ot
                    # proj pieces with batched psum->sbuf copies on the two
                    # free engines, three wide stores
                    so3 = sop.tile([P, C // P, QGW], B_DT, tag="so3", bufs=1,
                                   name="so3")
                    for qb in range(QB):
                        emit_normalize(pp, pq, qb, ppvt, pstg, use_act=True)
                        pool, tag = (psC, "c") if qb % 2 == 0 else (psPV, "pv")
                        tp = pool.tile([P, P], B_DT, tag=tag, name=f"tp{qb}")
                        nc.tensor.matmul(
                            tp, lhsT=pstg[:, qb, :], rhs=id_sb,
                            is_transpose=True, start=True, stop=True,
                        )
                        gc = slice(pq * QGW + qb * P, pq * QGW + (qb + 1) * P)
                        if qb % 2 == 0:
                            nc.scalar.copy(outh_sb[:, pp, gc], tp)
                        else:
                            nc.vector.tensor_copy(outh_sb[:, pp, gc], tp)
                    for ot in range(C // P):
                        pool, tag = (psC, "c") if ot % 2 == 0 else (psPV, "pv")
                        ps = pool.tile([P, QGW], f32, tag=tag, name=f"pjt{ot}")
                        for qb in range(QB):
                            gc = slice(pq * QGW + qb * P, pq * QGW + (qb + 1) * P)
                            for fc in range(FC):
                                nc.tensor.matmul(
                                    ps[:, qb * P : (qb + 1) * P],
                                    lhsT=wp_sb[:, fc, ot * P : (ot + 1) * P],
                                    rhs=outh_sb[:, fc, gc],
                                    start=(qb == 0 and fc == 0),
                                    stop=(qb == QB - 1 and fc == FC - 1),
                                    skip_group_check=True,
                                )
                        if ot % 2 == 0:
                            nc.scalar.copy(so3[:, ot, :], ps)
                        else:
                            nc.vector.tensor_copy(so3[:, ot, :], ps)
                        if ot == 1 or ot == 3:
                            eng = nc.sync
                            eng.dma_start(
                                out[(ot - 1) * P : (ot + 1) * P,
                                    pq * QGW : (pq + 1) * QGW].rearrange(
                                    "(b p) q -> p b q", p=P
                                ),
                                so3[:, ot - 1 : ot + 1, :],
                            )
                        elif ot >= 4:
                            eng = nc.sync
                            eng.dma_start(
                                out[ot * P : (ot + 1) * P,
                                    pq * QGW : (pq + 1) * QGW],
                                so3[:, ot, :],
                            )

            for si, (p_, qg) in enumerate(SEG_ORDER):
                LAG = 6 if si == 0 else 2
                units = due[si]
                total_cost = sum(u[0] for u in units) or 1e-9
                done_cost = 0.0
                ui = 0
                pvt = stg = None
                for ch in range(NT):
                    emit_scores(p_, qg, ch)
                    if ch == 0:
                        finish_segment(phase=0)
                    if ch == 1:
                        finish_segment(phase=1)
                        stg = stgp.tile(
                            [P, QB, 2 * D], B_DT, tag="stg", name=f"stg{si}"
                        )
                        pvt = [
                            psPV.tile([P, 512], f32, tag="pv", name=f"pv{si}_{t}")
                            for t in range(2)
                        ]
                    target = (ch + 1) / NT * total_cost
                    while ui < len(units) and done_cost < target:
                        cost, go, min_ch = units[ui]
                        if ch < min_ch:
                            break
                        go()
                        done_cost += cost
                        ui += 1
                    if ch >= LAG:
                        emit_pv(p_, qg, ch - LAG, pvt)
                for c in range(NT - LAG, NT - 1):
                    emit_pv(p_, qg, c, pvt)
                pending_end[0] = (p_, qg, pvt, stg, 1)
            finish_segment(phase=2)
    nc.compile()
    return nc


def _get_nc():
    global _CACHED_NC
    if _CACHED_NC is None:
        _CACHED_NC = build_nc()
    return _CACHED_NC


def shard_inputs(x, w_qkv, w_proj):
    """Build per-core input maps from full inputs (host side, bf16)."""
    in_maps = []
    for c in range(NCORES):
        b, g = divmod(c, 2)
        r = slice(HG * D * g, HG * D * (g + 1))

        def ptile(m):
            return np.ascontiguousarray(
                m.reshape(m.shape[0] // P, P, m.shape[1]).transpose(1, 0, 2)
            ).astype(NP_BF)

        xTc = ptile(x[b].T)
        wq = w_qkv[r]
        wk = w_qkv[C + HG * D * g : C + HG * D * (g + 1)]
        wv_ = w_qkv[2 * C + HG * D * g : 2 * C + HG * D * (g + 1)]
        wqk = ptile(np.concatenate([wq, wk], axis=0).T)
        wvT = ptile(wv_.T)
        wpT = ptile(w_proj[:, r].T)
        in_maps.append({"xT": xTc, "wqk": wqk, "wv": wvT, "wp": wpT})
    return in_maps


def run(x, w_qkv, w_proj, b_proj, trace=False):
    nc = _get_nc()
    in_maps = shard_inputs(x, w_qkv, w_proj)
    try:
        res = run_bass_kernel_spmd(nc, in_maps, list(range(NCORES)), trace=trace)
    except Exception:
        res = run_bass_kernel_spmd(nc, in_maps, list(range(NCORES)), trace=trace)
    y = np.empty((B, N, C), np.float32)
    for b in range(B):
        part = res.results[2 * b]["out"].astype(np.float32) + res.results[
            2 * b + 1
        ]["out"].astype(np.float32)
        y[b] = part.T + b_proj.astype(np.float32)
    return y, res


def kernel(x, w_qkv, w_proj, b_proj):
    x = np.asarray(x, dtype=np.float32)
    w_qkv = np.asarray(w_qkv, dtype=np.float32)
    w_proj = np.asarray(w_proj, dtype=np.float32)
    b_proj = np.asarray(b_proj, dtype=np.float32)
    y, _ = run(x, w_qkv, w_proj, b_proj, trace=False)
    return y


# revision 41
# speedup vs baseline: 1.0039x; 1.0039x over previous
"""Multi-head attention (B=4, N=2048, C=768, H=12) on 8 Trainium2 NeuronCores.

Sharding: core c = (batch b = c//2, head-group g = c%2 of 6 heads).
Each core: qkv projection for its (b, g), attention for 6 heads, partial
output projection against w_proj[:, g-cols]. Host sums the two partial
projections per batch, adds bias, transposes. No collectives.

v3 design: ACT exp stream (192 x [128,1024] activations ~= 199us) is the
bottleneck; PE work is ~185us thanks to transposed-PV (out [q=128, 65]
costs 65 rows/matmul instead of 512+). Everything is scheduled to keep
ACT streaming:
  - diagonal segment order (pair, qgroup) so early segments only need
    pair0's q/k/v and projections spread evenly
  - all deferred PE work (qkv pieces, v-chunks, proj) split into
    ~160-320ns units, deadline-paced 1-2 per chunk slot
  - all inputs bf16, loads n-ordered and fine-grained at the start
  - softmax denominators per-partition after transposed PV: normalize is
    reciprocal + tensor_scalar on DVE; [q,d]->[d,q] via dma transpose
  - last q-group's projection pre-accumulates fc0/fc1 into SBUF so the
    tail after the final normalize is one fc2 matmul + add per piece
PSUM (8 banks): scores [128,1024]x2 (4) + PV [128,512]x2 (2) + qkv/proj
[128,<=512]x2 (2).

Attention per segment (pair p, qgroup qg of 512 q), chunk ch (128 k):
  sT[k, q] e0|e1 -> one [128,1024] psum tile; one exp -> eT bf16 sbuf
  PV per (e, qb): lhsT = eT[:, e*512+qb*128 :+128], rhs = vT[:, ch, h*65 :+65]
  (col 64 of vT == 1 -> denominator lands at out[q, 64])
"""

import sys

for _p in ("/opt/trn_rl_repo", "/root/.axon_site/_ro/trn_rl_repo"):
    if _p not in sys.path:
        sys.path.insert(0, _p)

import numpy as np
import ml_dtypes

import concourse.bass as bass
import concourse.bacc as bacc
import concourse.mybir as mybir
import concourse.tile as tile
from concourse.bass_utils import run_bass_kernel_spmd

B, N, C = 4, 2048, 768
H, D = 12, 64
HG = 6          # heads per core
P = 128
NCORES = 8
KC = C // P     # 6 contraction chunks for qkv
NT = N // P     # 16 k-chunks of 128
NP_ = 3         # head pairs per core
QGW = 512       # q-group width
NQG = N // QGW  # 4
QB = QGW // P   # 4 q-blocks of 128 per q-group
FC = HG * D // P  # 3 proj contraction blocks
SCALE = D ** -0.5

B_DT = mybir.dt.bfloat16
NP_BF = ml_dtypes.bfloat16

# diagonal (pair, qgroup) order: early segments only need pair0 inputs,
# q-groups complete progressively so proj work spreads out
SEG_ORDER = [
    (0, 0), (0, 1), (1, 0), (0, 2), (1, 1), (2, 0),
    (0, 3), (1, 2), (2, 1), (1, 3), (2, 2), (2, 3),
]

_CACHED_NC = None
DEBUG_DUMPS = False


def build_nc():
    nc = bacc.Bacc("TRN2", target_bir_lowering=False, debug=False, num_devices=NCORES)
    f32 = mybir.dt.float32

    xT = nc.declare_dram_parameter("xT", [P, KC, N], B_DT, isOutput=False)
    wqk = nc.declare_dram_parameter("wqk", [P, KC, 2 * HG * D], B_DT, isOutput=False)
    wv = nc.declare_dram_parameter("wv", [P, KC, HG * D], B_DT, isOutput=False)
    wp = nc.declare_dram_parameter("wp", [P, FC, C], B_DT, isOutput=False)
    out = nc.declare_dram_parameter("out", [C, N], B_DT, isOutput=True)
    if DEBUG_DUMPS:
        dbg_qk = nc.declare_dram_parameter("dbg_qk", [P, 2 * HG * D // P, N], B_DT, isOutput=True)
        dbg_vt = nc.declare_dram_parameter("dbg_vt", [P, NT, HG * (D + 1)], B_DT, isOutput=True)
        dbg_outh = nc.declare_dram_parameter("dbg_outh", [P, FC, N], B_DT, isOutput=True)
        dbg_et = nc.declare_dram_parameter("dbg_et", [P, 2 * QGW], B_DT, isOutput=True)
        dbg_pv = nc.declare_dram_parameter("dbg_pv", [P, 512], mybir.dt.float32, isOutput=True)

    with tile.TileContext(nc) as tc:
        with (
            tc.tile_pool(name="big", bufs=1) as big,
            tc.tile_pool(name="et", bufs=10) as etp,
            tc.tile_pool(name="nrm", bufs=8) as nrm,
            tc.tile_pool(name="stg", bufs=4) as stgp,
            tc.tile_pool(name="so", bufs=8) as sop,
            tc.tile_pool(name="psS", bufs=2, space="PSUM") as psS,
            tc.tile_pool(name="psPV", bufs=2, space="PSUM") as psPV,
            tc.tile_pool(name="psC", bufs=2, space="PSUM") as psC,
        ):
            xT_sb = big.tile([P, KC, N], B_DT)
            wqk_sb = big.tile([P, KC, 2 * HG * D], B_DT)
            wv_sb = big.tile([P, KC, HG * D], B_DT)
            wp_sb = big.tile([P, FC, C], B_DT)

            # ---- loads: first-exp critical path first, fine-grained ----
            nc.sync.dma_start(wqk_sb[:, :, 0:P], wqk[:, :, 0:P])              # q pair0
            nc.scalar.dma_start(wqk_sb[:, :, 3 * P : 4 * P], wqk[:, :, 3 * P : 4 * P])  # k pair0
            for kc in range(KC):  # tokens 0:512 per contraction chunk
                eng = nc.sync if kc % 2 == 0 else nc.scalar
                eng.dma_start(xT_sb[:, kc, 0:QGW], xT[:, kc, 0:QGW])
            nc.scalar.dma_start(wv_sb, wv[:, :, :])
            for npc in range(1, NQG):
                nc.sync.dma_start(
                    xT_sb[:, :, npc * QGW : (npc + 1) * QGW],
                    xT[:, :, npc * QGW : (npc + 1) * QGW],
                )
            nc.sync.dma_start(wqk_sb[:, :, P : 3 * P], wqk[:, :, P : 3 * P])
            nc.sync.dma_start(wqk_sb[:, :, 4 * P :], wqk[:, :, 4 * P :])
            nc.sync.dma_start(wp_sb, wp[:, :, :])

            # warm the ACT exp table during the load phase
            warm = nrm.tile([1, 32], f32, tag="warm")
            nc.vector.memset(warm, 0.0)
            nc.scalar.activation(warm, warm, mybir.ActivationFunctionType.Exp,
                                 bias=0.0, scale=1.0)
            # warm the PE p-state: junk matmuls bridge the input-load latency
            # so real work starts at full clock with no ramp resets
            junk = big.tile([P, D], B_DT, name="junk")
            nc.vector.memset(junk, 0.0)
            junk_ps = psC.tile([D, D], f32, tag="c", name="junk_ps")
            for _ in range(100):
                nc.tensor.matmul(junk_ps, lhsT=junk, rhs=junk, start=True, stop=True)
            # 128x128 identity (bf16) for PE-side transposes in the endgame
            io_a = big.tile([P, P], mybir.dt.float32, name="io_a")
            io_b = big.tile([P, 1], mybir.dt.float32, name="io_b")
            nc.gpsimd.iota(io_a, pattern=[[1, P]], base=0, channel_multiplier=0,
                           allow_small_or_imprecise_dtypes=True)
            nc.gpsimd.iota(io_b, pattern=[[0, 1]], base=0, channel_multiplier=1,
                           allow_small_or_imprecise_dtypes=True)
            id_sb = big.tile([P, P], B_DT, name="id_sb")
            nc.vector.tensor_scalar(id_sb, io_a, io_b[:, 0:1], None,
                                    op0=mybir.AluOpType.is_equal)

            qk_sb = big.tile([P, 2 * HG * D // P, N], B_DT)
            vT_sb = big.tile([P, NT, HG * (D + 1)], B_DT)
            ones_view = vT_sb.rearrange("p n (h s) -> p n h s", s=D + 1)[:, :, :, D : D + 1]
            nc.vector.memset(ones_view, 1.0)
            outh_sb = big.tile([P, FC, N], B_DT)

            # ---------------- deferred work units (~160-320 ns each) -----
            def u_qkv(blk, nf):
                """qk_sb[:, blk, nf*128:+128] (128-token piece, 6 matmuls)"""
                def go():
                    ps = psC.tile([P, P], f32, tag="c", name=f"qkv_ps{blk}_{nf}")
                    for kc in range(KC):
                        nc.tensor.matmul(
                            ps,
                            lhsT=wqk_sb[:, kc, blk * P : (blk + 1) * P],
                            rhs=xT_sb[:, kc, nf * P : (nf + 1) * P],
                            start=(kc == 0),
                            stop=(kc == KC - 1),
                        )
                    nc.vector.tensor_copy(qk_sb[:, blk, nf * P : (nf + 1) * P], ps)
                return (0.32, go, 0)

            def u_vt(p_, nt):
                """v rows for pair p_, token chunk nt -> vT_sb (6 matmuls)"""
                def go():
                    ps = psC.tile([P, 2 * D], f32, tag="c", name=f"vt_ps{p_}_{nt}")
                    for kc in range(KC):
                        nc.tensor.matmul(
                            ps,
                            lhsT=xT_sb[:, kc, nt * P : (nt + 1) * P],
                            rhs=wv_sb[:, kc, 2 * p_ * D : 2 * (p_ + 1) * D],
                            start=(kc == 0),
                            stop=(kc == KC - 1),
                        )
                    nc.vector.tensor_copy(
                        vT_sb.rearrange("p n (h s) -> p n h s", s=D + 1)[
                            :, nt, 2 * p_ : 2 * (p_ + 1), 0:D
                        ],
                        ps.rearrange("p (h s) -> p h s", s=D),
                    )
                return (0.32, go, 0)

            so_tiles = {}


            def u_proj(ot, qg, qb, min_ch=0):
                """projection piece [128, 128]: all 3 fc blocks + stage"""
                def go():
                    key = (ot, qg)
                    if key not in so_tiles:
                        so_tiles[key] = sop.tile(
                            [P, QGW], B_DT, tag="so", name=f"so{ot}_{qg}"
                        )
                    so = so_tiles[key]
                    ps = psC.tile([P, P], f32, tag="c", name=f"pj_ps{ot}_{qg}_{qb}")
                    gcols = slice(qg * QGW + qb * P, qg * QGW + (qb + 1) * P)
                    for fc in range(FC):
                        nc.tensor.matmul(
                            ps,
                            lhsT=wp_sb[:, fc, ot * P : (ot + 1) * P],
                            rhs=outh_sb[:, fc, gcols],
                            start=(fc == 0),
                            stop=(fc == FC - 1),
                        )
                    nc.vector.tensor_copy(so[:, qb * P : (qb + 1) * P], ps)
                    if qb == QB - 1:
                        nc.sync.dma_start(
                            out[ot * P : (ot + 1) * P, qg * QGW : (qg + 1) * QGW], so
                        )
                return (0.16, go, min_ch)

            # ---------------- attention pieces ----------------
            et_tiles = {}

            def emit_scores(p_, qg, ch):
                sc = psS.tile([P, 2 * QGW], f32, tag="s", name=f"sc{p_}_{qg}_{ch}")
                for e in range(2):
                    base = e * D
                    nc.tensor.matmul(
                        sc[:, e * QGW : (e + 1) * QGW],
                        lhsT=qk_sb[base : base + D, NP_ + p_, ch * P : (ch + 1) * P],
                        rhs=qk_sb[base : base + D, p_, qg * QGW : (qg + 1) * QGW],
                        start=True,
                        stop=True,
                    )
                eT = etp.tile([P, 2 * QGW], B_DT, tag="et", name=f"et{p_}_{qg}_{ch}")
                nc.scalar.activation(
                    eT, sc, mybir.ActivationFunctionType.Exp,
                    bias=0.0, scale=float(SCALE),
                )
                if DEBUG_DUMPS and (p_, qg, ch) == (0, 0, 0):
                    nc.sync.dma_start(dbg_et[:, :], eT)
                et_tiles[(p_, qg, ch)] = eT

            def emit_pv(p_, qg, ch, pvt):
                # one psum zero-region (bank) per pvt tile: only the first
                # region write starts the group, only the last stops it
                eT = et_tiles.pop((p_, qg, ch))
                for e in range(2):
                    h = 2 * p_ + e
                    for qb in range(QB):
                        t, sub = divmod(qb, 2)
                        nc.tensor.matmul(
                            pvt[t][:, sub * 256 + e * (D + 1) : sub * 256 + (e + 1) * (D + 1)],
                            lhsT=eT[:, e * QGW + qb * P : e * QGW + (qb + 1) * P],
                            rhs=vT_sb[:, ch, h * (D + 1) : (h + 1) * (D + 1)],
                            start=(ch == 0 and e == 0 and sub == 0),
                            stop=(ch == NT - 1 and e == 1 and sub == 1),
                            skip_group_check=True,
                        )

            def emit_normalize(p_, qg, qb, pvt, stg, use_act=False):
                t, sub = divmod(qb, 2)
                seg_tile = pvt[t][:, sub * 256 : sub * 256 + 2 * (D + 1)]
                den = seg_tile.rearrange("p (e s) -> p e s", s=D + 1)[:, :, D]
                rc = nrm.tile([P, 2], f32, tag="rc", name=f"rc{p_}_{qg}_{qb}")
                nc.vector.reciprocal(rc, den)
                for e in range(2):
                    if use_act and e == 0:
                        nc.scalar.activation(
                            stg[:, qb, e * D : (e + 1) * D],
                            seg_tile[:, e * (D + 1) : e * (D + 1) + D],
                            mybir.ActivationFunctionType.Copy,
                            bias=0.0, scale=rc[:, e : e + 1],
                        )
                    else:
                        nc.vector.tensor_scalar_mul(
                            stg[:, qb, e * D : (e + 1) * D],
                            seg_tile[:, e * (D + 1) : e * (D + 1) + D],
                            rc[:, e : e + 1],
                        )

            # ---------------- deadline-paced filler schedule --------------
            due = {si: [] for si in range(len(SEG_ORDER))}
            # seg0: k(p0) token pieces n1..15 + vt(p0) interleaved slot-wise,
            # then q(p0,qg1)
            s0 = [u_qkv(NP_ + 0, i) for i in range(1, 4)]
            s0.append(u_qkv(0, 4))                               # q(p0,qg1) piece
            for i in range(NT):
                s0.append(u_vt(0, i))
                if 4 + i < NT:
                    s0.append(u_qkv(NP_ + 0, 4 + i))
            s0 += [u_qkv(0, 5 + i) for i in range(3)]            # q(p0,qg1) rest
            due[0] = s0
            due[1] = (
                [u_qkv(1, i) for i in range(4)]                  # q(p1,qg0)
                + [u_qkv(NP_ + 1, i) for i in range(NT)]         # k(p1)
                + [u_vt(1, 0)]
            )
            due[2] = (
                [u_vt(1, i) for i in range(1, NT)]
                + [u_qkv(0, 8 + i) for i in range(4)]            # q(p0,qg2)
            )
            due[3] = (
                [u_qkv(1, 4 + i) for i in range(4)]              # q(p1,qg1)
                + [u_qkv(2, i) for i in range(4)]                # q(p2,qg0)
                + [u_qkv(NP_ + 2, i) for i in range(NT)]         # k(p2)
            )
            due[4] = (
                [u_vt(2, i) for i in range(NT)]
                + [u_qkv(0, 12 + i) for i in range(4)]           # q(p0,qg3)
            )
            due[5] = (
                [u_qkv(1, 8 + i) for i in range(4)]              # q(p1,qg2)
                + [u_qkv(2, 4 + i) for i in range(4)]            # q(p2,qg1)
            )
            due[6] = (
                [u_qkv(1, 12 + i) for i in range(4)]             # q(p1,qg3)
                + [u_proj(ot, 0, qb, 4) for ot in range(3) for qb in range(QB)]
            )
            due[7] = (
                [u_qkv(2, 8 + i) for i in range(4)]              # q(p2,qg2)
                + [u_proj(ot, 0, qb) for ot in range(3, 6) for qb in range(QB)]
            )
            due[8] = [u_qkv(2, 12 + i) for i in range(4)]        # q(p2,qg3)
            due[9] = [u_proj(ot, 1, qb, 4) for ot in range(3) for qb in range(QB)]
            due[10] = [u_proj(ot, 1, qb) for ot in range(3, 6) for qb in range(QB)]
            due[11] = [u_proj(ot, 2, qb, 4) for ot in range(6) for qb in range(QB)]

            # ---------------- upfront: q(p0,qg0) + k(p0,ch0) --------------
            ps_q = psC.tile([P, QGW], f32, tag="c", name="up_q")
            ps_k = psC.tile([P, P], f32, tag="c", name="up_k")
            for kc in range(KC):
                nc.tensor.matmul(
                    ps_q,
                    lhsT=wqk_sb[:, kc, 0:P],
                    rhs=xT_sb[:, kc, 0:QGW],
                    start=(kc == 0),
                    stop=(kc == KC - 1),
                )
                nc.tensor.matmul(
                    ps_k,
                    lhsT=wqk_sb[:, kc, NP_ * P : (NP_ + 1) * P],
                    rhs=xT_sb[:, kc, 0:P],
                    start=(kc == 0),
                    stop=(kc == KC - 1),
                )
            nc.vector.tensor_copy(qk_sb[:, 0, 0:QGW], ps_q)
            nc.scalar.copy(qk_sb[:, NP_, 0:P], ps_k)

            # ---------------- main loop ----------------
            pending_end = [None]

            def finish_segment(phase=2):
                """Previous segment's endgame, split across the next segment's
                first two chunk slots so the ACT stream never waits on
                boundary work."""
                if pending_end[0] is None:
                    return
                pp, pq, ppvt, pstg, plag = pending_end[0]
                if phase == 0:
                    for c in range(NT - plag, NT - 1):
                        emit_pv(pp, pq, c, ppvt)
                    return
                pending_end[0] = None
                if phase == 2:
                    for c in range(NT - plag, NT - 1):
                        emit_pv(pp, pq, c, ppvt)
                emit_pv(pp, pq, NT - 1, ppvt)
                if DEBUG_DUMPS and (pp, pq) == (0, 0):
                    pvstg = big.tile([P, 512], f32, name="pvstg")
                    nc.vector.tensor_copy(pvstg, ppvt[0])
                    nc.sync.dma_start(dbg_pv[:, :], pvstg)
                if (pp, pq) != (2, 3):
                    for qb in range(QB):
                        emit_normalize(pp, pq, qb, ppvt, pstg)
                    nc.sync.dma_start_transpose(
                        outh_sb[:, pp, pq * QGW : (pq + 1) * QGW].rearrange(
                            "p (b q) -> p b q", q=P
                        ),
                        pstg.rearrange("p b f -> p (b f)"),
                    )
                else:
                    # endgame: PE-side transposes (no DMA latency), per-ot
                    # proj pieces with batched psum->sbuf copies on the two
                    # free engines, three wide stores
                    so3 = sop.tile([P, C // P, QGW], B_DT, tag="so3", bufs=1,
                                   name="so3")
                    for qb in range(QB):
                        emit_normalize(pp, pq, qb, ppvt, pstg, use_act=True)
                        pool, tag = (psC, "c") if qb % 2 == 0 else (psPV, "pv")
                        tp = pool.tile([P, P], B_DT, tag=tag, name=f"tp{qb}")
                        nc.tensor.matmul(
                            tp, lhsT=pstg[:, qb, :], rhs=id_sb,
                            is_transpose=True, start=True, stop=True,
                        )
                        gc = slice(pq * QGW + qb * P, pq * QGW + (qb + 1) * P)
                        if qb % 2 == 0:
                            nc.scalar.copy(outh_sb[:, pp, gc], tp)
                        else:
                            nc.vector.tensor_copy(outh_sb[:, pp, gc], tp)
                    for ot in range(C // P):
                        pool, tag = (psC, "c") if ot % 2 == 0 else (psPV, "pv")
                        ps = pool.tile([P, QGW], f32, tag=tag, name=f"pjt{ot}")
                        for qb in range(QB):
                            gc = slice(pq * QGW + qb * P, pq * QGW + (qb + 1) * P)
                            for fc in range(FC):
                                nc.tensor.matmul(
                                    ps[:, qb * P : (qb + 1) * P],
                                    lhsT=wp_sb[:, fc, ot * P : (ot + 1) * P],
                                    rhs=outh_sb[:, fc, gc],
                                    start=(qb == 0 and fc == 0),
                                    stop=(qb == QB - 1 and fc == FC - 1),
                                    skip_group_check=True,
                                )
                        if ot % 2 == 0:
                            nc.scalar.copy(so3[:, ot, :], ps)
                        else:
                            nc.vector.tensor_copy(so3[:, ot, :], ps)
                        if ot == 1 or ot == 3:
                            eng = nc.sync
                            eng.dma_start(
                                out[(ot - 1) * P : (ot + 1) * P,
                                    pq * QGW : (pq + 1) * QGW].rearrange(
                                    "(b p) q -> p b q", p=P
                                ),
                                so3[:, ot - 1 : ot + 1, :],
                            )
                        elif ot >= 4:
                            eng = nc.sync
                            eng.dma_start(
                                out[ot * P : (ot + 1) * P,
                                    pq * QGW : (pq + 1) * QGW],
                                so3[:, ot, :],
                            )

            for si, (p_, qg) in enumerate(SEG_ORDER):
                LAG = 6 if si == 0 else 4
                units = due[si]
                total_cost = sum(u[0] for u in units) or 1e-9
                done_cost = 0.0
                ui = 0
                pvt = stg = None
                for ch in range(NT):
                    emit_scores(p_, qg, ch)
                    if ch == 0:
                        finish_segment(phase=0)
                    if ch == 1:
                        finish_segment(phase=1)
                        stg = stgp.tile(
                            [P, QB, 2 * D], B_DT, tag="stg", name=f"stg{si}"
                        )
                        pvt = [
                            psPV.tile([P, 512], f32, tag="pv", name=f"pv{si}_{t}")
                            for t in range(2)
                        ]
                    target = (ch + 1) / NT * total_cost
                    while ui < len(units) and done_cost < target:
                        cost, go, min_ch = units[ui]
                        if ch < min_ch:
                            break
                        go()
                        done_cost += cost
                        ui += 1
                    if ch >= LAG:
                        emit_pv(p_, qg, ch - LAG, pvt)
                for c in range(NT - LAG, NT - 1):
                    emit_pv(p_, qg, c, pvt)
                pending_end[0] = (p_, qg, pvt, stg, 1)
            finish_segment(phase=2)
    nc.compile()
    return nc


def _get_nc():
    global _CACHED_NC
    if _CACHED_NC is None:
        _CACHED_NC = build_nc()
    return _CACHED_NC


def shard_inputs(x, w_qkv, w_proj):
    """Build per-core input maps from full inputs (host side, bf16)."""
    in_maps = []
    for c in range(NCORES):
        b, g = divmod(c, 2)
        r = slice(HG * D * g, HG * D * (g + 1))

        def ptile(m):
            return np.ascontiguousarray(
                m.reshape(m.shape[0] // P, P, m.shape[1]).transpose(1, 0, 2)
            ).astype(NP_BF)

        xTc = ptile(x[b].T)
        wq = w_qkv[r]
        wk = w_qkv[C + HG * D * g : C + HG * D * (g + 1)]
        wv_ = w_qkv[2 * C + HG * D * g : 2 * C + HG * D * (g + 1)]
        wqk = ptile(np.concatenate([wq, wk], axis=0).T)
        wvT = ptile(wv_.T)
        wpT = ptile(w_proj[:, r].T)
        in_maps.append({"xT": xTc, "wqk": wqk, "wv": wvT, "wp": wpT})
    return in_maps


def run(x, w_qkv, w_proj, b_proj, trace=False):
    nc = _get_nc()
    in_maps = shard_inputs(x, w_qkv, w_proj)
    try:
        res = run_bass_kernel_spmd(nc, in_maps, list(range(NCORES)), trace=trace)
    except Exception:
        res = run_bass_kernel_spmd(nc, in_maps, list(range(NCORES)), trace=trace)
    y = np.empty((B, N, C), np.float32)
    for b in range(B):
        part = res.results[2 * b]["out"].astype(np.float32) + res.results[
            2 * b + 1
        ]["out"].astype(np.float32)
        y[b] = part.T + b_proj.astype(np.float32)
    return y, res


def kernel(x, w_qkv, w_proj, b_proj):
    x = np.asarray(x, dtype=np.float32)
    w_qkv = np.asarray(w_qkv, dtype=np.float32)
    w_proj = np.asarray(w_proj, dtype=np.float32)
    b_proj = np.asarray(b_proj, dtype=np.float32)
    y, _ = run(x, w_qkv, w_proj, b_proj, trace=False)
    return y
